# revision 7
# baseline (speedup 1.0000x reference)
"""GraphSAGE GNN Bass kernel for TRN2, 8-core SPMD — v2.

Strategy (dst-partitioned, SBUF-resident segment sums):
  - Core c owns dst nodes [c*V, (c+1)*V). Node-major bf16 feature table
    [NC*VP, H] replicated in every core's HBM, rebuilt per layer by AllGather.
  - Phase A (per dst-chunk D, per src-bucket b): in-edges grouped into
    128-slot windows (dst-sorted, <=32 segs/window, no seg split). HBM
    dma_gather fetches h[src] slot-major (4096 idx/call sweet spot);
    per window a PE matmul with one-hot M [128,32] segment-sums into PSUM
    (4 windows/PSUM tile at quadrants); full-group copies land seg rows
    bf16 in SBUF seg_sb[b] (row w*32+j -> partition r%128, rank r//128).
  - Combine (per chunk): SBUF-source transpose dma_gather pulls each dst's
    <=NB seg rows feature-major; 3 adds + inv_deg mul -> agg bf16.
  - Transform: z = Wl^T agg + Wr^T hT in PSUM; BN stats accumulated,
    AllReduced; affine(+ReLU) -> hT bf16; PE transposes stage node-major
    bf16 into agin, AllGather -> next layer table. Classifier per core.
"""

import numpy as np
import ml_dtypes
import concourse.bass as bass
import concourse.tile as tile
from concourse import bacc, mybir
from concourse.masks import make_identity

F32 = mybir.dt.float32
BF16 = mybir.dt.bfloat16
I16 = mybir.dt.int16

NC = 8
NB = 4
H = 128
W = 32          # max segs per window
ND = 10         # dst chunks per core
PIECE_W = 32    # windows per gather piece
GC = 2          # dsts per combine-gather descriptor (elem = GC*256B)
EPS = 1e-5


class Cfg:
    def __init__(self, N=100000, E=3200000, d_in=12):
        self.N, self.E, self.d_in = N, E, d_in
        self.V = N // NC                      # 12500
        self.VP = ((self.V + 127) // 128) * 128   # 12544
        self.NT = self.VP // 128              # 98
        self.TROWS = NC * self.VP
        self.BROWS = self.TROWS // NB         # 25088
        assert self.BROWS <= 32768
        # chunk tile counts (sum = NT), tapered so late chunks shrink and
        # their combine/transform hides under the next chunk's gathers
        self.chunk_tiles = [13, 13, 13, 12, 12, 11, 9, 7, 5, 3]
        assert sum(self.chunk_tiles) == self.NT
        self.chunk_tile0 = np.concatenate([[0], np.cumsum(self.chunk_tiles)])


def _wrap16(flat):
    assert flat.size % 16 == 0
    return np.tile(np.ascontiguousarray(flat.reshape(-1, 16).T), (8, 1))


def _pack_group(src_rows, dst_l, invd_l, d0, ndD):
    """Pack one (core, chunk, bucket) edge group with quad-aligned segs.
    Iterates ALL dst ids [d0, d0+ndD) in quads of 4 (empty dsts get
    zero-slot segs). Windows: <=128 slots, <=32 segs, break only at quad
    boundaries. Returns slot_rows [nw*128], mm [128, nw*W] (values
    inv_deg[dst]), quadrow [ndD/4] (seg row of quad start / 4), nw."""
    order = np.argsort(dst_l, kind="stable")
    s, d = src_rows[order], dst_l[order]
    deg = np.bincount(d - d0, minlength=ndD)
    start = np.concatenate([[0], np.cumsum(deg)])
    nq = ndD // GC
    qs_all = deg.reshape(nq, GC).sum(1)
    assert qs_all.max(initial=0) <= 128
    # first-fit decreasing: assign quads to windows (<=128 slots, <=W segs)
    order_q = np.argsort(-qs_all, kind="stable")
    win_fill, win_segs, win_quads = [], [], []
    for q in order_q:
        qs = int(qs_all[q])
        placed = False
        for wi in range(len(win_fill)):
            if win_fill[wi] + qs <= 128 and win_segs[wi] + GC <= W:
                win_fill[wi] += qs
                win_segs[wi] += GC
                win_quads[wi].append(q)
                placed = True
                break
        if not placed:
            win_fill.append(qs)
            win_segs.append(GC)
            win_quads.append([q])
    quadrow = np.zeros(nq, np.int64)
    w_of = np.zeros(ndD, np.int64)
    j_of = np.zeros(ndD, np.int64)
    start_of = np.zeros(ndD, np.int64)
    for wi, quads in enumerate(win_quads):
        fill, segs = 0, 0
        for q in quads:
            quadrow[q] = (wi * W + segs) // GC
            for r in range(GC):
                dd = q * GC + r
                w_of[dd], j_of[dd], start_of[dd] = wi, segs, wi * 128 + fill
                fill += int(deg[dd])
                segs += 1
    cur_w = len(win_quads) - 1 if win_quads else 0
    nw = cur_w + 1
    slot_rows = np.zeros(nw * 128, np.int64)
    slot_j = np.full(nw * 128, -1, np.int64)
    slot_d = np.full(nw * 128, -1, np.int64)
    dl0 = d - d0
    pos = start_of[dl0] + (np.arange(d.size) - start[dl0])
    slot_rows[pos] = s
    slot_j[pos] = j_of[dl0]
    slot_d[pos] = d
    sl = np.nonzero(slot_j >= 0)[0]
    mm = np.zeros((128, nw * W), np.float32)
    mm[sl % 128, (sl // 128) * W + slot_j[sl]] = invd_l[slot_d[sl]]
    return slot_rows, mm, quadrow, nw


def preprocess(edge_index, cfg: Cfg):
    src = np.asarray(edge_index[0], np.int64)
    dst = np.asarray(edge_index[1], np.int64)
    N, V, VP = cfg.N, cfg.V, cfg.VP
    deg = np.bincount(dst, minlength=N).astype(np.float32)
    inv_deg = (np.float32(1.0) / np.maximum(deg, np.float32(1.0))).astype(np.float32)

    def table_row(u):
        return (u // V) * VP + (u % V)

    rows_g = table_row(src)
    buck = rows_g // cfg.BROWS
    rows_b = rows_g - buck * cfg.BROWS
    core_of = dst // V
    dst_l = dst - core_of * V
    tile_of = dst_l // 128
    t2c = np.zeros(cfg.NT, np.int64)
    for D in range(ND):
        t2c[cfg.chunk_tile0[D]:cfg.chunk_tile0[D + 1]] = D
    chunk_of = t2c[tile_of]

    packs = {}
    nwin = np.zeros((NC, ND, NB), np.int64)
    for c in range(NC):
        mc = core_of == c
        rb_c, dl_c, bk_c, ck_c = rows_b[mc], dst_l[mc], buck[mc], chunk_of[mc]
        invd_c = np.zeros(VP, np.float32)
        invd_c[:V] = inv_deg[c * V:(c + 1) * V]
        for D in range(ND):
            mD = ck_c == D
            d0 = int(cfg.chunk_tile0[D]) * 128
            ndD = cfg.chunk_tiles[D] * 128
            for b in range(NB):
                m2 = mD & (bk_c == b)
                p = _pack_group(rb_c[m2], dl_c[m2], invd_c, d0, ndD)
                packs[(c, D, b)] = p
                nwin[c, D, b] = p[3]

    # uniform window counts: max over cores, rounded to multiple of 4
    NWIN = [[int(-(-nwin[:, D, b].max() // 4) * 4) for b in range(NB)]
            for D in range(ND)]
    pieces = [[None] * NB for _ in range(ND)]
    for D in range(ND):
        for b in range(NB):
            nwt, pl = NWIN[D][b], []
            while nwt > 0:
                pw = min(PIECE_W, nwt)
                pl.append(pw)
                nwt -= pw
            pieces[D][b] = pl
    TOTW = sum(NWIN[D][b] for D in range(ND) for b in range(NB))
    NWINMAX = max(NWIN[D][b] for D in range(ND) for b in range(NB))

    cid_cols = sum(cfg.chunk_tiles[D] * 128 * 4 // GC // 16 for D in range(ND))
    pre = dict(NWIN=NWIN, pieces=pieces, TOTW=TOTW, NWINMAX=NWINMAX,
               inv_deg=inv_deg, cid_cols=cid_cols,
               gidx=[], mm=[], cidx=[])
    for c in range(NC):
        gidx = np.zeros((128, TOTW * 8), np.int16)
        mm = np.zeros((128, TOTW * W), ml_dtypes.bfloat16)
        cidx = np.zeros((128, cid_cols), np.int16)
        woff = 0
        coff = 0
        for D in range(ND):
            ndD = cfg.chunk_tiles[D] * 128
            nix = ndD * 4 // GC
            qv = np.zeros(nix, np.int64)
            for b in range(NB):
                slot_rows, mmat, quadrow, nw = packs[(c, D, b)]
                nwp = NWIN[D][b]
                sr = np.zeros(nwp * 128, np.int64)
                sr[:slot_rows.size] = slot_rows
                gidx[:, woff * 8:(woff + nwp) * 8] = _wrap16(sr.astype(np.int16))
                mm[:, woff * W:woff * W + mmat.shape[1]] = mmat.astype(ml_dtypes.bfloat16)
                # gather slot i = q*4 + b -> GC-group index into segarr[D]
                qv[b::4] = quadrow + b * (NWINMAX * 32 // GC)
                woff += nwp
            cidx[:, coff:coff + nix // 16] = _wrap16(qv.astype(np.int16))
            coff += nix // 16
        pre["gidx"].append(gidx)
        pre["mm"].append(mm)
        pre["cidx"].append(cidx)
    return pre


def build_inputs(inputs, pre, cfg: Cfg):
    N, V, VP, d_in = cfg.N, cfg.V, cfg.VP, cfg.d_in
    x = np.asarray(inputs["x"], np.float32)
    tbl0 = np.zeros((cfg.TROWS, H), ml_dtypes.bfloat16)
    for c in range(NC):
        tbl0[c * VP:c * VP + V, :d_in] = x[c * V:(c + 1) * V]

    def padT(w, rows, cols):
        o = np.zeros((rows, cols), np.float32)
        o[:w.shape[0], :w.shape[1]] = w
        return o

    Wl0 = np.asarray(inputs["Wl0"], np.float32)
    Wr0 = np.asarray(inputs["Wr0"], np.float32)
    Wl = np.asarray(inputs["Wl"], np.float32)
    Wr = np.asarray(inputs["Wr"], np.float32)
    wlT = np.stack([padT(Wl0.T, H, H), Wl[0].T, Wl[1].T]).astype(ml_dtypes.bfloat16)
    wrT = np.stack([padT(Wr0.T, H, H), Wr[0].T, Wr[1].T]).astype(ml_dtypes.bfloat16)
    gam = np.ascontiguousarray(np.asarray(inputs["gamma"], np.float32).T)
    bet = np.ascontiguousarray(np.asarray(inputs["beta"], np.float32).T)
    wc1T = np.ascontiguousarray(np.asarray(inputs["Wc1"], np.float32).T).astype(ml_dtypes.bfloat16)
    bc1 = np.asarray(inputs["bc1"], np.float32).reshape(-1, 1)
    wc2T = np.ascontiguousarray(np.asarray(inputs["Wc2"], np.float32).T).astype(ml_dtypes.bfloat16)
    bc2 = np.asarray(inputs["bc2"], np.float32).reshape(1, 1)
    lsel = np.zeros((4, 128, 128), ml_dtypes.bfloat16)
    pp = np.arange(128)
    for s in range(4):
        u, r = s // GC, s % GC
        lsel[s, pp, u * (128 // (4 // GC)) + (pp // 4) * GC + r] = 1.0

    in_maps = []
    for c in range(NC):
        xT = np.zeros((128, VP), ml_dtypes.bfloat16)
        xT[:d_in, :V] = x[c * V:(c + 1) * V].T
        in_maps.append(dict(
            tbl0=tbl0, xT=xT,
            gidx=pre["gidx"][c], mm=pre["mm"][c], cidx=pre["cidx"][c],
            lsel=lsel, wlT=wlT, wrT=wrT, gam=gam, bet=bet,
            wc1T=wc1T, bc1=bc1, wc2T=wc2T, bc2=bc2,
        ))
    return in_maps


def build_program(cfg: Cfg, pre, layers=3, dbg=None, no_coll=False,
                  skip_gather=False, skip_m=False):
    N, V, VP, NT = cfg.N, cfg.V, cfg.VP, cfg.NT
    NWIN, pieces = pre["NWIN"], pre["pieces"]
    NWINMAX, TOTW = pre["NWINMAX"], pre["TOTW"]
    NDCMAX = max(cfg.chunk_tiles) * 128

    nc = bacc.Bacc("TRN2", target_bir_lowering=False, debug=False, num_devices=NC)

    tbl0 = nc.dram_tensor("tbl0", [cfg.TROWS, H], BF16, kind="ExternalInput")
    xT_e = nc.dram_tensor("xT", [128, VP], BF16, kind="ExternalInput")
    gidx_e = nc.dram_tensor("gidx", [128, TOTW * 8], I16, kind="ExternalInput")
    mm_e = nc.dram_tensor("mm", [128, TOTW * W], BF16, kind="ExternalInput")
    cidx_e = nc.dram_tensor("cidx", [128, pre["cid_cols"]], I16, kind="ExternalInput")
    lsel_e = nc.dram_tensor("lsel", [4, 128, 128], BF16, kind="ExternalInput")
    wlT_e = nc.dram_tensor("wlT", [3, H, H], BF16, kind="ExternalInput")
    wrT_e = nc.dram_tensor("wrT", [3, H, H], BF16, kind="ExternalInput")
    gam_e = nc.dram_tensor("gam", [H, 3], F32, kind="ExternalInput")
    bet_e = nc.dram_tensor("bet", [H, 3], F32, kind="ExternalInput")
    wc1T_e = nc.dram_tensor("wc1T", [H, 64], BF16, kind="ExternalInput")
    bc1_e = nc.dram_tensor("bc1", [64, 1], F32, kind="ExternalInput")
    wc2T_e = nc.dram_tensor("wc2T", [64, 1], BF16, kind="ExternalInput")
    bc2_e = nc.dram_tensor("bc2", [1, 1], F32, kind="ExternalInput")
    logits_e = nc.dram_tensor("logits", [1, VP], F32, kind="ExternalOutput")
    dbg_e = nc.dram_tensor("dbg", [128, VP], F32, kind="ExternalOutput") if dbg else None

    tbls = [tbl0,
            nc.dram_tensor("tbl1", [cfg.TROWS, H], BF16, addr_space="Shared"),
            nc.dram_tensor("tbl2", [cfg.TROWS, H], BF16, addr_space="Shared")]
    agin = [None,
            nc.dram_tensor("agin1", [VP, H], BF16),
            nc.dram_tensor("agin2", [VP, H], BF16)]
    segarr = [nc.dram_tensor(f"segarr{i}", [NB, NWINMAX * 32, H], BF16)
              for i in range(2)]
    arin = [nc.dram_tensor(f"arin{l}", [H, 2], F32) for l in range(3)]
    arout = [nc.dram_tensor(f"arout{l}", [H, 2], F32, addr_space="Shared")
             for l in range(3)]
    rg = [list(range(NC))]

    with tile.TileContext(nc) as tc:
        import contextlib
        cm = contextlib.ExitStack()
        with cm:
            singles = cm.enter_context(tc.tile_pool(name="singles", bufs=1))
            persist = cm.enter_context(tc.tile_pool(name="persist", bufs=1))
            stagp = cm.enter_context(tc.tile_pool(name="stagp", bufs=4))
            fpool = cm.enter_context(tc.tile_pool(name="fpool", bufs=4))
            mpool = cm.enter_context(tc.tile_pool(name="mpool", bufs=4))
            cpool = cm.enter_context(tc.tile_pool(name="cpool", bufs=3))
            small = cm.enter_context(tc.tile_pool(name="small", bufs=4))
            ps_seg = cm.enter_context(tc.tile_pool(name="ps_seg", bufs=4, space="PSUM"))
            ps_tf = cm.enter_context(tc.tile_pool(name="ps_tf", bufs=2, space="PSUM"))
            ps_tr = cm.enter_context(tc.tile_pool(name="ps_tr", bufs=1, space="PSUM"))

            # ---- constants ----
            wlT = singles.tile([H, 3, H], BF16, tag="wlT")
            wrT = singles.tile([H, 3, H], BF16, tag="wrT")
            nc.sync.dma_start(out=wlT[:], in_=wlT_e[:].rearrange("l k m -> k l m"))
            nc.sync.dma_start(out=wrT[:], in_=wrT_e[:].rearrange("l k m -> k l m"))
            gam = singles.tile([H, 3], F32, tag="gam")
            bet = singles.tile([H, 3], F32, tag="bet")
            nc.sync.dma_start(out=gam[:], in_=gam_e[:])
            nc.sync.dma_start(out=bet[:], in_=bet_e[:])
            wc1T = singles.tile([H, 64], BF16, tag="wc1T")
            nc.sync.dma_start(out=wc1T[:], in_=wc1T_e[:])
            bc1 = singles.tile([64, 1], F32, tag="bc1")
            nc.sync.dma_start(out=bc1[:], in_=bc1_e[:])
            wc2T = singles.tile([64, 1], BF16, tag="wc2T")
            nc.sync.dma_start(out=wc2T[:], in_=wc2T_e[:])
            bc2 = singles.tile([1, 1], F32, tag="bc2")
            nc.sync.dma_start(out=bc2[:], in_=bc2_e[:])
            lsel = singles.tile([128, 4, 128], BF16, tag="lsel")
            nc.sync.dma_start(out=lsel[:], in_=lsel_e[:].rearrange("r k m -> k r m"))
            identb = singles.tile([128, 128], BF16, tag="identb")
            make_identity(nc, identb[:])
            epsT = singles.tile([128, 1], F32, tag="epsT")
            nc.vector.memset(epsT[:], EPS)

            fdummy = mdummy = None
            if skip_gather:
                fdummy = singles.tile([128, PIECE_W, 128], BF16, tag="fdummy")
                nc.vector.memset(fdummy[:], 0.001)
            if skip_m:
                mdummy = singles.tile([128, PIECE_W * W], BF16, tag="mdummy")
                nc.vector.memset(mdummy[:], 0.001)
            hT = persist.tile([128, VP], BF16, tag="hT")
            nc.sync.dma_start(out=hT[:], in_=xT_e[:])
            zt = persist.tile([128, VP], BF16, tag="zt")

            n_tf_tot = sum((cfg.chunk_tiles[D] * 128 + 511) // 512 for D in range(ND))

            def phase_a(layer, D):
                tbl = tbls[layer]
                seg_d = segarr[D % 2]
                woff = sum(NWIN[Dp][bp] for Dp in range(D) for bp in range(NB))
                for b in range(NB):
                    tbl_b = tbl[b * cfg.BROWS:(b + 1) * cfg.BROWS, :]
                    goff = 0
                    for pw in pieces[D][b]:
                        if skip_m:
                            m_t = mdummy
                        else:
                            m_t = mpool.tile([128, PIECE_W * W], BF16, tag="m")
                            nc.scalar.dma_start(out=m_t[:, :pw * W],
                                                in_=mm_e[:, woff * W:(woff + pw) * W])
                        g_t = mpool.tile([128, PIECE_W * 8], I16, tag="g")
                        nc.scalar.dma_start(out=g_t[:, :pw * 8],
                                            in_=gidx_e[:, woff * 8:(woff + pw) * 8])
                        if skip_gather:
                            f_t = fdummy
                        else:
                            f_t = fpool.tile([128, PIECE_W, 128], BF16, tag="f")
                            nc.gpsimd.dma_gather(
                                out_ap=f_t[:, :pw, :], in_ap=tbl_b,
                                idxs_ap=g_t[:, :pw * 8],
                                num_idxs=pw * 128, num_idxs_reg=pw * 128,
                                elem_size=H, single_packet=False)
                        ng = pw // 4
                        stag = stagp.tile([128, PIECE_W // 4, 128], BF16,
                                          tag="stag")
                        for g in range(ng):
                            pseg = ps_seg.tile([128, 128], F32, tag="pseg")
                            for k in range(4):
                                w = g * 4 + k
                                nc.tensor.matmul(
                                    pseg[k * W:(k + 1) * W, :],
                                    m_t[:, w * W:(w + 1) * W],
                                    f_t[:, w, :],
                                    start=True, stop=True,
                                    tile_position=(0, k * W))
                            nc.vector.tensor_copy(out=stag[:, g, :], in_=pseg[:])
                        r0 = goff * 128
                        nc.sync.dma_start(
                            out=seg_d[b, r0:r0 + ng * 128, :]
                            .rearrange("(g p) f -> p g f", p=128),
                            in_=stag[:, :ng, :])
                        goff += ng
                        woff += pw

            def combine_transform(layer, D, sums, sumsq, tfi0):
                ndD = cfg.chunk_tiles[D] * 128
                d0 = int(cfg.chunk_tile0[D]) * 128
                seg_d = segarr[D % 2]
                nix = ndD * 4 // GC
                coff = sum(cfg.chunk_tiles[Dp] * 128 * 4 // GC // 16
                           for Dp in range(D))
                ci = cpool.tile([128, NDCMAX * 4 // GC // 16], I16, tag="ci")
                nc.sync.dma_start(out=ci[:, :nix // 16],
                                  in_=cidx_e[:, coff:coff + nix // 16])
                go = cpool.tile([128, NDCMAX * 4 // GC // 128, GC * 128], BF16,
                                tag="go")
                nc.gpsimd.dma_gather(
                    out_ap=go[:, :nix // 128, :], in_ap=seg_d[:].rearrange(
                        "b (q x) f -> (b q) (x f)", x=GC),
                    idxs_ap=ci[:, :nix // 16],
                    num_idxs=nix, num_idxs_reg=nix, elem_size=GC * H,
                    single_packet=False)
                agg = cpool.tile([128, NDCMAX], BF16, tag="agg")
                for t in range(ndD // 128):
                    pagg = ps_seg.tile([128, 128], F32, tag="pseg")
                    for s in range(4):
                        u, r = s // GC, s % GC
                        nc.tensor.matmul(pagg[:], lsel[:, s, :],
                                         go[:, t * (4 // GC) + u,
                                            r * 128:(r + 1) * 128],
                                         start=(s == 0), stop=(s == 3))
                    ptr = ps_tr.tile([128, 128], BF16, tag="trp")
                    atile = cpool.tile([128, 128], BF16, tag="atile")
                    nc.vector.tensor_copy(out=atile[:], in_=pagg[:])
                    nc.tensor.transpose(out=ptr[:], in_=atile[:],
                                        identity=identb[:])
                    nc.vector.tensor_copy(out=agg[:, t * 128:(t + 1) * 128],
                                          in_=ptr[:])
                if dbg == "AGG" and layer == 0:
                    nc.vector.tensor_copy(out=dbgt[:, d0:d0 + ndD], in_=agg[:, :ndD])
                # transform
                ti = tfi0
                for c0 in range(0, ndD, 512):
                    ntc = min(512, ndD - c0)
                    pz = ps_tf.tile([128, 512], F32, tag="pz")
                    nc.tensor.matmul(pz[:, :ntc], wlT[:, layer, :],
                                     agg[:, c0:c0 + ntc], start=True, stop=False)
                    nc.tensor.matmul(pz[:, :ntc], wrT[:, layer, :],
                                     hT[:, d0 + c0:d0 + c0 + ntc], start=False, stop=True)
                    nc.vector.tensor_copy(out=zt[:, d0 + c0:d0 + c0 + ntc],
                                          in_=pz[:, :ntc])
                    nc.vector.reduce_sum(out=sums[:, ti:ti + 1], in_=pz[:, :ntc],
                                         axis=mybir.AxisListType.X)
                    sq = cpool.tile([128, 512], F32, tag="sq")
                    nc.vector.tensor_mul(sq[:, :ntc], zt[:, d0 + c0:d0 + c0 + ntc],
                                         zt[:, d0 + c0:d0 + c0 + ntc])
                    nc.vector.reduce_sum(out=sumsq[:, ti:ti + 1], in_=sq[:, :ntc],
                                         axis=mybir.AxisListType.X)
                    ti += 1
                return ti

            if dbg in ("AGG", "Z", "H"):
                dbgt = persist.tile([128, VP], F32, tag="dbgt")

            for layer in range(layers):
                sums = small.tile([128, n_tf_tot], F32, tag="sums")
                sumsq = small.tile([128, n_tf_tot], F32, tag="sumsq")
                tfi = 0
                for D in range(ND):
                    phase_a(layer, D)
                    if D > 0:
                        tfi = combine_transform(layer, D - 1, sums, sumsq, tfi)
                tfi = combine_transform(layer, ND - 1, sums, sumsq, tfi)

                stats2 = small.tile([128, 2], F32, tag="stats2")
                nc.vector.reduce_sum(out=stats2[:, 0:1], in_=sums[:],
                                     axis=mybir.AxisListType.X)
                nc.vector.reduce_sum(out=stats2[:, 1:2], in_=sumsq[:],
                                     axis=mybir.AxisListType.X)
                gstat = small.tile([128, 2], F32, tag="gstat")
                if no_coll:
                    nc.vector.tensor_scalar_mul(gstat[:], stats2[:], float(NC))
                else:
                    nc.sync.dma_start(out=arin[layer][:], in_=stats2[:])
                    nc.gpsimd.collective_compute(
                        "AllReduce", mybir.AluOpType.add, replica_groups=rg,
                        ins=[arin[layer][:]], outs=[arout[layer][:]])
                    nc.sync.dma_start(out=gstat[:], in_=arout[layer][:])
                mean = small.tile([128, 1], F32, tag="mean")
                va = small.tile([128, 1], F32, tag="va")
                aa = small.tile([128, 1], F32, tag="aa")
                cc = small.tile([128, 1], F32, tag="cc")
                nc.vector.tensor_scalar_mul(mean[:], gstat[:, 0:1], 1.0 / N)
                nc.vector.tensor_scalar_mul(va[:], gstat[:, 1:2], 1.0 / N)
                nc.vector.tensor_mul(cc[:], mean[:], mean[:])
                nc.vector.tensor_sub(va[:], va[:], cc[:])
                nc.scalar.activation(out=va[:], in_=va[:],
                                     func=mybir.ActivationFunctionType.Sqrt,
                                     bias=epsT[:], scale=1.0)
                nc.vector.reciprocal(va[:], va[:])
                nc.vector.tensor_mul(aa[:], gam[:, layer:layer + 1], va[:])
                nc.vector.tensor_mul(cc[:], mean[:], aa[:])
                nc.vector.tensor_sub(cc[:], bet[:, layer:layer + 1], cc[:])

                for c0 in range(0, VP, 512):
                    nt = min(512, VP - c0)
                    if layer < 2:
                        nc.scalar.activation(out=hT[:, c0:c0 + nt],
                                             in_=zt[:, c0:c0 + nt],
                                             func=mybir.ActivationFunctionType.Relu,
                                             bias=cc[:], scale=aa[:])
                    else:
                        nc.vector.tensor_scalar(out=hT[:, c0:c0 + nt],
                                                in0=zt[:, c0:c0 + nt],
                                                scalar1=aa[:], scalar2=cc[:],
                                                op0=mybir.AluOpType.mult,
                                                op1=mybir.AluOpType.add)
                if dbg == "H" and layer == layers - 1:
                    nc.vector.tensor_copy(out=dbgt[:], in_=hT[:])
                if dbg == "Z" and layer == layers - 1:
                    nc.vector.tensor_copy(out=dbgt[:], in_=zt[:])

                if layer < 2 and layers == 3 and no_coll:
                    for t2 in range(NT):
                        ptr = ps_tr.tile([128, 128], BF16, tag="trp")
                        nc.tensor.transpose(out=ptr[:],
                                            in_=hT[:, t2 * 128:(t2 + 1) * 128],
                                            identity=identb[:])
                        nc.vector.tensor_copy(out=zt[:, t2 * 128:(t2 + 1) * 128],
                                              in_=ptr[:])
                    nc.sync.dma_start(
                        out=agin[layer + 1][:].rearrange("(t p) f -> p t f", p=128),
                        in_=zt[:].rearrange("p (t f) -> p t f", f=128))
                    nc.sync.dma_start(
                        out=tbls[layer + 1][cfg.VP:2 * cfg.VP, :],
                        in_=agin[layer + 1][:])
                if layer < 2 and layers == 3 and not no_coll:
                    for t2 in range(NT):
                        ptr = ps_tr.tile([128, 128], BF16, tag="trp")
                        nc.tensor.transpose(out=ptr[:],
                                            in_=hT[:, t2 * 128:(t2 + 1) * 128],
                                            identity=identb[:])
                        nc.vector.tensor_copy(out=zt[:, t2 * 128:(t2 + 1) * 128],
                                              in_=ptr[:])
                    nc.sync.dma_start(
                        out=agin[layer + 1][:].rearrange("(t p) f -> p t f", p=128),
                        in_=zt[:].rearrange("p (t f) -> p t f", f=128))
                    nc.gpsimd.collective_compute(
                        "AllGather", mybir.AluOpType.bypass, replica_groups=rg,
                        ins=[agin[layer + 1][:]], outs=[tbls[layer + 1][:]])

            # classifier
            for c0 in range(0, VP, 512):
                nt = min(512, VP - c0)
                pc1 = ps_tf.tile([128, 512], F32, tag="pz")
                nc.tensor.matmul(pc1[:64, :nt], wc1T[:], hT[:, c0:c0 + nt],
                                 start=True, stop=True)
                h3 = small.tile([64, 512], BF16, tag="h3")
                nc.scalar.activation(out=h3[:, :nt], in_=pc1[:64, :nt],
                                     func=mybir.ActivationFunctionType.Relu,
                                     bias=bc1[:], scale=1.0)
                pc2 = ps_tr.tile([1, 512], F32, tag="pc2")
                nc.tensor.matmul(pc2[:, :nt], wc2T[:], h3[:, :nt],
                                 start=True, stop=True)
                lsb = small.tile([1, 512], F32, tag="lsb")
                nc.vector.tensor_scalar_add(lsb[:, :nt], pc2[:, :nt], bc2[:])
                nc.sync.dma_start(out=logits_e[:, c0:c0 + nt], in_=lsb[:, :nt])
            if dbg:
                nc.sync.dma_start(out=dbg_e[:], in_=dbgt[:])

    nc.compile()
    return nc


# ======================= harness entry points =======================
def _run_with_retry(nc, in_maps, cores, tries=3):
    from concourse.bass_utils import run_bass_kernel_spmd
    last = None
    for _ in range(tries):
        try:
            return run_bass_kernel_spmd(nc, in_maps, cores)
        except Exception as e:  # transient axon terminal failures
            last = e
    raise last


def kernel(**inputs):
    """Full-input entry: shards across 8 NeuronCores internally."""
    cfg = Cfg()
    edge_index = np.asarray(inputs["edge_index"])
    pre = preprocess(edge_index, cfg)
    in_maps = build_inputs(inputs, pre, cfg)
    nc = build_program(cfg, pre)
    res = _run_with_retry(nc, in_maps, list(range(NC)))
    logits = np.concatenate(
        [np.asarray(res.results[c]["logits"])[0, :cfg.V] for c in range(NC)]
    ).astype(np.float32)
    return logits


def benchmark(inputs, reps=30, nc=None, pre=None, in_maps=None):
    """Steady-state device-resident timing: N back-to-back dispatches with a
    single host sync at the end (device queue stays full, so per-call time is
    device execution, not the ~80 ms axon per-dispatch round-trip floor).
    Returns (per_call_ns, logits)."""
    import time
    import jax
    from jax.sharding import Mesh, PartitionSpec, NamedSharding
    from jax.experimental.shard_map import shard_map
    from concourse import bass2jax

    cfg = Cfg()
    if pre is None:
        pre = preprocess(np.asarray(inputs["edge_index"]), cfg)
    if in_maps is None:
        in_maps = build_inputs(inputs, pre, cfg)
    if nc is None:
        nc = build_program(cfg, pre)
    bass2jax.install_neuronx_cc_hook()
    n_cores = NC
    in_names, out_names, out_avals, zero_outs = [], [], [], []
    for alloc in nc.m.functions[0].allocations:
        if not isinstance(alloc, mybir.MemoryLocationSet):
            continue
        name = alloc.memorylocations[0].name
        if alloc.kind == "ExternalInput":
            if nc.partition_id_tensor is not None and name == nc.partition_id_tensor.name:
                continue
            in_names.append(name)
        elif alloc.kind == "ExternalOutput":
            shape = tuple(alloc.tensor_shape)
            dtype = mybir.dt.np(alloc.dtype)
            out_names.append(name)
            out_avals.append(jax.core.ShapedArray(shape, dtype))
            zero_outs.append(np.zeros(shape, dtype))
    n_params = len(in_names)
    all_in_names = in_names + out_names
    if nc.partition_id_tensor is not None:
        all_in_names.append(nc.partition_id_tensor.name)

    def _body(*args):
        ops = list(args)
        if nc.partition_id_tensor is not None:
            ops.append(bass2jax.partition_id_tensor())
        return tuple(bass2jax._bass_exec_p.bind(
            *ops, out_avals=tuple(out_avals), in_names=tuple(all_in_names),
            out_names=tuple(out_names), lowering_input_output_aliases=(),
            sim_require_finite=False, sim_require_nnan=False, nc=nc))

    mesh = Mesh(np.asarray(jax.devices()[:n_cores]), ("core",))
    sharded = jax.jit(shard_map(_body, mesh=mesh,
                                in_specs=(PartitionSpec("core"),) * (n_params + len(out_names)),
                                out_specs=(PartitionSpec("core"),) * len(out_names),
                                check_rep=False),
                      keep_unused=True)
    sh = NamedSharding(mesh, PartitionSpec("core"))
    dev_in = [jax.device_put(np.concatenate(
        [np.asarray(in_maps[c][nm])[None] for c in range(n_cores)], axis=0), sh)
        for nm in in_names]
    zeros = [jax.device_put(np.zeros((n_cores, *z.shape), z.dtype), sh)
             for z in zero_outs]
    for d in dev_in + zeros:
        d.block_until_ready()
    out = sharded(*dev_in, *zeros)
    for o in out:
        o.block_until_ready()
    # fill the dispatch pipeline, then time one long unbroken window ending in
    # a full drain (any intermediate sync would re-pay the ~80 ms dispatch
    # round trip; intermediate-future timestamps under-report on this path)
    outs = [sharded(*dev_in, *zeros) for _ in range(8)]
    for o in outs[-1]:
        o.block_until_ready()
    t0 = time.time()
    outs = [sharded(*dev_in, *zeros) for _ in range(reps)]
    for o in outs[-1]:
        o.block_until_ready()
    per_call = (time.time() - t0) / reps
    print(f"steady-state per-call: {per_call*1e3:.2f} ms over {reps} reps")
    out = outs[-1]
    est_ns = per_call * 1e9
    la = np.asarray(out[out_names.index("logits")]).reshape(n_cores, 1, cfg.VP)
    logits = np.concatenate([la[c, 0, :cfg.V] for c in range(n_cores)]).astype(np.float32)
    return est_ns, logits
